# revision 24
# baseline (speedup 1.0000x reference)
"""Trainium2 Bass kernel for DendriticFullyConnected.

Math (B=128, IN=OUT=1024):
    state = sigmoid(x @ W_non.T + b_non) - 1
    syn   = x[:,None,:] * W_nmda[None,:,:]            # [B,O,I]
    clus  = 0.5*(syn[...,:-1] + syn[...,1:])          # conv [0.5,0.5]
    ca    = relu(clus.sum(-1))
    n     = 2 + state;  kd = 0.5**n;  xn = ca**n
    out   = xn/(kd+xn) + state

Key algebraic collapses:
  1. conv+sum == dot product against W_nmda with first/last columns halved
     -> the [B,O,I] tensor never exists; the module is two matmuls.
  2. Hill fraction in log space with s = sigmoid(zn), n = 1+s:
         out = sigmoid(n * ln(2*relu(zm) + eps)) + s - 1
     (GPSIMD pow would shorten the tail but is software-emulated on the
     Q7 cores: measured ~10us per 16K-element op. ACT exp/ln only.)

Sharding: OUT split 8 ways (128 outputs/core), x replicated. Per-core HBM
traffic ~1.1MB bf16; every weight byte is read exactly once chip-wide.

Per-core schedule:
  - Two PSUM groups: zn = x@Wn.T + b (8 K-chunks + 2-row hi/lo bias
    matmul), then zm = x@Wm.T. zn completes first, so the state sigmoid
    (ACT exp + DVE add/recip) overlaps the zm matmuls; by the time zm
    lands only the 4-op Pool tail remains.
  - DMAs split across queues (SP: x halves + wm0; Pool: wn halves + wm1)
    so transfers overlap and the first K-chunks land ~2x earlier;
    matmuls start while the rest of the stream is still in flight.
  - PE warmup matmuls at t~0 keep the HAM clock ramping under the DMA
    shadow; activation-table pass pinned to one set -> single table load
    at t=0, also under the DMA shadow.
"""

import numpy as np

_B, _IN, _OUT, _NC = 128, 1024, 1024, 8
_OSH = _OUT // _NC  # 128 outputs per core
_KT = _IN // 128    # 8 contraction chunks
_MMDT = "bfloat16"

_PIN_ACT_SET = "natural_log_exp_and_others"

_CFG = {
    "nw": 6,            # PE warmup matmuls (64-col)
    "split_x": False,   # x in 2 chunks
    "split_wn": False,  # wn stream in 2 chunks
    "split_wm": False,  # wm in 2 chunks
    "x_eng": "sp",      # issuing engine per stream: sp | pool
    "wn_eng": "sp",
    "wm_eng": "sp",
    "wm1_eng": "sp",    # second wm half when split_wm
    "wn_dtype": "bfloat16",  # W_non stream dtype (bfloat16 | float8e4)
    "wm_dtype": "bfloat16",  # W_nmda stream dtype
    "zm_split": False,  # zm in 2 col-halves: tail pipelines across ACT/DVE
    "bufs": 1,          # tile-pool buffers (2 = double-buffer across loop iters)
    "merge_xwn": False, # x + wn streams as ONE DMA (requires bf16 wn)
}

_state = {}


def _rearr(m):
    # [128 rows, 1024 cols] -> out[p, j*128 + r] = m[r, j*128 + p]
    # per 128-col chunk j: contraction index on partitions, row index free.
    return np.ascontiguousarray(
        m.reshape(128, _KT, 128).transpose(2, 1, 0).reshape(128, _IN)
    )


def _make_bacc_cls():
    import concourse.bacc as bacc
    import concourse.mybir as mybir
    from concourse.hw_specs import get_activation_tables
    import bass_rust as _bass_rust

    class PinnedActBacc(bacc.Bacc):
        """Force all activations onto one table set: one ACT table load."""

        def insert_act_table_loads(self):
            has_activation = any(
                isinstance(i, mybir.InstActivation)
                for b in self.main_func.blocks
                for i in b.instructions
            )
            if not has_activation:
                return
            tables = list(get_activation_tables(self.m.arch).items())
            names = [t[0] for t in tables]
            if _PIN_ACT_SET not in names:
                _bass_rust.insert_act_table_loads(self, tables)
                return
            canon = names.index(_PIN_ACT_SET)
            keep = [tables[canon]]
            _bass_rust.insert_act_table_loads(self, keep)
            for b in self.main_func.blocks:
                for i in b.instructions:
                    if isinstance(i, mybir.InstLoadActFuncSet):
                        i.act_func_set_id = canon

    return PinnedActBacc


def _build(loop_n=None, cfg=None):
    cfg = {**_CFG, **(cfg or {})}
    import concourse.mybir as mybir
    import concourse.tile as tile
    from concourse.bass import ts
    from concourse.bass_utils import run_bass_kernel_spmd

    dt = mybir.dt.float32
    mdt = getattr(mybir.dt, _MMDT)
    AF = mybir.ActivationFunctionType
    OP = mybir.AluOpType

    nc = _make_bacc_cls()(
        "TRN2",
        target_bir_lowering=False,
        debug=False,
        enable_asserts=False,
        num_devices=_NC,
    )
    wndt = getattr(mybir.dt, cfg["wn_dtype"])
    wmdt = getattr(mybir.dt, cfg["wm_dtype"])
    # wn stream: 8 K-chunks of W_non + 128 bias cols (p0=b_hi, p1=b_lo)
    if cfg["merge_xwn"]:
        assert cfg["wn_dtype"] == _MMDT
        xwn = nc.dram_tensor(
            "xwn", [128, 2 * _IN + _OSH], mdt, kind="ExternalInput"
        ).ap()
    else:
        xT = nc.dram_tensor("xT", [128, _IN], mdt, kind="ExternalInput").ap()
        wnb = nc.dram_tensor(
            "wnb", [128, _IN + _OSH], wndt, kind="ExternalInput"
        ).ap()
    wm = nc.dram_tensor("wm", [128, _IN], wmdt, kind="ExternalInput").ap()
    out = nc.dram_tensor("out", [_B, _OSH], dt, kind="ExternalOutput").ap()

    NW = cfg["nw"]
    WNW = _IN + _OSH  # 1152

    def body(tc, io, ep, ps):
        # tiny memsets first: warmup source, bias lhsT, pow(-1) exponent,
        # ACT-warm input
        wsrc = io.tile([2, 128], mybir.dt.bfloat16)
        nc.vector.memset(wsrc[:], 0.0)
        ones = io.tile([2, _B], mybir.dt.bfloat16)
        nc.vector.memset(ones[:], 1.0)
        warm0 = ep.tile([1, 1], dt)
        nc.vector.memset(warm0[:], 1.0)
        # eps doubles as the u >= -42.5 clamp: ln(3.36e-19) = -42.5 keeps
        # t = n*u >= -85 so exp(-t) stays finite (n < 2)
        eps = ep.tile([128, 1], dt)
        nc.vector.memset(eps[:], 3.36e-19)
        # ACT table warm: pulls the single exp-set load to t~0.
        warm1 = ep.tile([1, 1], dt)
        nc.scalar.activation(warm1[:], warm0[:], AF.Exp)

        # DMAs: SP carries x halves + wm0 + out; Pool carries wn + wm1.
        wmt = io.tile([128, _IN], wmdt, name="wmt")
        eng = {"sp": nc.sync, "pool": nc.gpsimd}
        xe, wne, wme, wm1e = (
            eng[cfg["x_eng"]], eng[cfg["wn_eng"]], eng[cfg["wm_eng"]],
            eng[cfg["wm1_eng"]],
        )
        if cfg["merge_xwn"]:
            xwnt = io.tile([128, 2 * _IN + _OSH], mdt, name="xwnt")
            xe.dma_start(out=xwnt[:], in_=xwn[:])
            xt, wnt, xoff, wnoff = xwnt, xwnt, 0, _IN
        else:
            xt = io.tile([128, _IN], mdt, name="xt")
            wnt = io.tile([128, WNW], wndt, name="wnt")
            xoff, wnoff = 0, 0
        if cfg["merge_xwn"]:
            pass
        elif cfg["split_x"]:
            xe.dma_start(out=xt[:, 0:512], in_=xT[:, 0:512])
            xe.dma_start(out=xt[:, 512:1024], in_=xT[:, 512:1024])
        else:
            xe.dma_start(out=xt[:], in_=xT[:])
        if cfg["merge_xwn"]:
            pass
        elif cfg["split_wn"]:
            wne.dma_start(out=wnt[:, 0:512], in_=wnb[:, 0:512])
            wne.dma_start(out=wnt[:, 512:WNW], in_=wnb[:, 512:WNW])
        else:
            wne.dma_start(out=wnt[:], in_=wnb[:])
        if cfg["split_wm"]:
            wme.dma_start(out=wmt[:, 0:512], in_=wm[:, 0:512])
            wm1e.dma_start(out=wmt[:, 512:1024], in_=wm[:, 512:1024])
        else:
            wme.dma_start(out=wmt[:], in_=wm[:])

        # PE: warmups ramp the clock while DMAs land
        wp = ps.tile([128, 64], dt)
        for k in range(NW):
            nc.tensor.matmul(
                wp[:], wsrc[:], wsrc[:, 0:64],
                start=(k == 0), stop=(k == NW - 1),
            )

        # group 1: zn = x @ Wn.T + b
        zn = ps.tile([_B, _OSH], dt)
        for j in range(_KT):
            nc.tensor.matmul(
                zn[:],
                xt[:, xoff + j * 128 : xoff + (j + 1) * 128],
                wnt[:, wnoff + j * 128 : wnoff + (j + 1) * 128],
                start=(j == 0), stop=False,
            )
        nc.tensor.matmul(
            zn[:], ones[:], wnt[0:2, wnoff + _IN : wnoff + _IN + _OSH],
            start=False, stop=True
        )
        # group 2: zm = x @ Wm.T (optionally as 2 col-halves so the tail
        # on the first half pipelines under the second half's matmuls)
        if cfg["zm_split"]:
            zms = [ps.tile([_B, 64], dt, name=f"zm{h}") for h in range(2)]
            for h in range(2):
                for j in range(_KT):
                    nc.tensor.matmul(
                        zms[h][:],
                        xt[:, xoff + j * 128 : xoff + (j + 1) * 128],
                        wmt[:, j * 128 + 64 * h : j * 128 + 64 * (h + 1)],
                        start=(j == 0), stop=(j == _KT - 1),
                    )
        else:
            zm = ps.tile([_B, _OSH], dt)
            for j in range(_KT):
                nc.tensor.matmul(
                    zm[:],
                    xt[:, xoff + j * 128 : xoff + (j + 1) * 128],
                    wmt[:, ts(j, 128)],
                    start=(j == 0), stop=(j == _KT - 1),
                )
            zms = [zm]

        # state sigmoid (overlaps zm matmuls): s = 1/(1+exp(-zn))
        e0 = ep.tile([_B, _OSH], dt)
        nc.scalar.activation(e0[:], zn[:], AF.Exp, scale=-1.0)
        d0 = ep.tile([_B, _OSH], dt)
        nc.vector.tensor_scalar_add(d0[:], e0[:], 1.0)
        s = ep.tile([_B, _OSH], dt)
        nc.vector.reciprocal_approx_fast(s[:], d0[:])

        # tail: u = ln(2*relu(zm)+eps) on ACT (relu then ln, same engine);
        # eps keeps t = n*u >= -85 so exp(-t) stays finite (n < 2).
        # t = (s+1)*u;  y = sigmoid(t) = 1/(1+exp(-t));  res = y + s - 1
        res = ep.tile([_B, _OSH], dt)
        if cfg["zm_split"]:
            halves = [(zms[h][:], slice(64 * h, 64 * (h + 1))) for h in range(2)]
        else:
            halves = [(zms[0][:], slice(0, _OSH))]
        us, tts, e1s = [], [], []
        for i, (zmh, sl) in enumerate(halves):
            w = sl.stop - sl.start
            rl = ep.tile([_B, w], dt, name=f"rl{i}")
            nc.scalar.activation(rl[:], zmh, AF.Relu)
            u = ep.tile([_B, w], dt, name=f"u{i}")
            nc.scalar.activation(u[:], rl[:], AF.Ln, scale=2.0, bias=eps[:])
            us.append(u)
        for i, (zmh, sl) in enumerate(halves):
            w = sl.stop - sl.start
            t = ep.tile([_B, w], dt, name=f"t{i}")
            nc.vector.scalar_tensor_tensor(
                t[:], s[:, sl], 1.0, us[i][:], OP.add, OP.mult
            )
            tts.append(t)
        for i, (zmh, sl) in enumerate(halves):
            w = sl.stop - sl.start
            e1 = ep.tile([_B, w], dt, name=f"e1{i}")
            nc.scalar.activation(e1[:], tts[i][:], AF.Exp, scale=-1.0)
            e1s.append(e1)
        for i, (zmh, sl) in enumerate(halves):
            w = sl.stop - sl.start
            d1 = ep.tile([_B, w], dt, name=f"d1{i}")
            nc.vector.tensor_scalar_add(d1[:], e1s[i][:], 1.0)
            y = ep.tile([_B, w], dt, name=f"y{i}")
            nc.vector.reciprocal_approx_fast(y[:], d1[:])
            nc.vector.scalar_tensor_tensor(
                res[:, sl], y[:], -1.0, s[:, sl], OP.add, OP.add
            )
        nc.sync.dma_start(out=out[:], in_=res[:])

    with tile.TileContext(nc) as tc:
        with (
            tc.tile_pool(name="io", bufs=cfg["bufs"]) as io,
            tc.tile_pool(name="ep", bufs=cfg["bufs"]) as ep,
            tc.tile_pool(name="ps", bufs=cfg["bufs"], space="PSUM") as ps,
        ):
            if loop_n is None:
                body(tc, io, ep, ps)
            else:
                with tc.For_i(0, loop_n, 1):
                    body(tc, io, ep, ps)

    nc.compile()
    return nc, run_bass_kernel_spmd


def _prep_in_maps(inputs, W_nmda, W_non, b_non, cfg=None):
    import ml_dtypes

    cfg = {**_CFG, **(cfg or {})}
    npdt = ml_dtypes.bfloat16
    _np_w = {"float8e4": ml_dtypes.float8_e4m3, "bfloat16": ml_dtypes.bfloat16}
    wndt = _np_w[cfg["wn_dtype"]]
    wmdt = _np_w[cfg["wm_dtype"]]

    x = np.ascontiguousarray(np.asarray(inputs, dtype=np.float32))
    Wn = np.asarray(W_non, dtype=np.float32)
    Wm = np.asarray(W_nmda, dtype=np.float32).copy()
    Wm[:, 0] *= 0.5
    Wm[:, -1] *= 0.5
    b = np.asarray(b_non, dtype=np.float32)
    # bias as two hi/lo rows: bh + bl reproduces b to ~eps^2
    bh = b.astype(wndt).astype(np.float32)
    bl = b - bh

    xr = _rearr(x).astype(npdt)
    in_maps = []
    for c in range(_NC):
        sl = slice(c * _OSH, (c + 1) * _OSH)
        wnb = np.zeros((128, _IN + _OSH), np.float32)
        wnb[:, :_IN] = _rearr(Wn[sl])
        wnb[0, _IN:] = bh[sl]
        wnb[1, _IN:] = bl[sl]
        wmr = _rearr(Wm[sl])
        if cfg["merge_xwn"]:
            xwn = np.concatenate(
                [xr.astype(np.float32), wnb], axis=1
            ).astype(npdt)
            in_maps.append({"xwn": xwn, "wm": wmr.astype(wmdt)})
        else:
            in_maps.append(
                {
                    "xT": xr,
                    "wnb": wnb.astype(wndt),
                    "wm": wmr.astype(wmdt),
                }
            )
    return in_maps


def kernel(inputs, W_nmda, W_non, b_non):
    if "nc" not in _state:
        _state["nc"], _state["run"] = _build()
    nc, run = _state["nc"], _state["run"]
    in_maps = _prep_in_maps(inputs, W_nmda, W_non, b_non)
    res = run(nc, in_maps, list(range(_NC)))
    outs = res.results
    return np.concatenate([outs[c]["out"] for c in range(_NC)], axis=1)
